# revision 1
# baseline (speedup 1.0000x reference)
"""SRP layer distributed Bass kernel for TRN2 (v6).

Math (full problem): out = Psi_c @ x.T @ x with Psi_c = Psi - rowmean(Psi).
  x [D, N] f32, Psi [O, N] f32, out [O, N] f32  (D=4096, N=8192, O=2048)

Distribution over 8 cores as a 2x4 grid: core c -> (i = c % 2: n-half,
j = c // 2: o-quarter). Per core:
  x_i  [D, NL]    (NL = N/2)
  psi_ji [OL, NL] (OL = O/4)
  rs   [OT, 128]  host-precomputed psi_ji.sum(axis=1), o-tile-major rows
  xrs  [1, D]     host-precomputed x_i.sum(axis=1)
  out_ji [OL, NL]

Key structure (TensorE-facing data bf16 via SWDGE cast-DMA, PSUM f32):
  - mm1 runs on UNCENTERED psi; centering is a rank-1 correction
    tmp -= mean[o] * xrs_local[d] applied as K=1 matmuls into the same
    PSUM accumulation group (mean from a tiny pair-AllReduce of rs that
    runs off the critical path).
  - mm1 by d-chunks of 512: x rows cast-loaded bf16, PE-transposed into
    xT; next chunk's transposes interleave between mm groups.
  - tmp halves pair-AllReduced in bf16, overlapped with mm1 tail and
    mm2 pass A (kd lower half).
  - mm2 streams the natural-layout bf16 x copy written during mm1.
"""

from contextlib import ExitStack

import concourse.bacc as bacc
import concourse.mybir as mybir
import concourse.tile as tile
from concourse.masks import make_identity

F32 = mybir.dt.float32
BF = mybir.dt.bfloat16


def build_srp_kernel(
    D=4096,
    NL=4096,
    OL=512,
    NTOT=8192,
    n_cores=8,
    groups=((0, 1), (2, 3), (4, 5), (6, 7)),
    ar_dtype=BF,
):
    OT = OL // 128      # o-tiles
    KN = NL // 128      # n-tiles (mm1 contraction)
    DC = D // 512       # d-chunks (mm1 output cols)
    ND = NL // 512      # n-chunks (mm2 output cols)
    KD = D // 128       # d-tiles (mm2 contraction)
    DH = D // 2         # half of d (AR chunk)
    assert DC % 2 == 0 and KD % 2 == 0

    groups = [list(g) for g in groups]

    nc = bacc.Bacc("TRN2", target_bir_lowering=False, debug=False,
                   num_devices=n_cores)
    x_ext = nc.dram_tensor("x", [D, NL], F32, kind="ExternalInput")
    psi_ext = nc.dram_tensor("psi", [OL, NL], F32, kind="ExternalInput")
    rs_ext = nc.dram_tensor("rs", [1, OL], F32, kind="ExternalInput")
    xrs_ext = nc.dram_tensor("xrs", [1, D], F32, kind="ExternalInput")
    out_ext = nc.dram_tensor("out", [OL, NL], F32, kind="ExternalOutput")

    with ExitStack() as stack:
        tc = stack.enter_context(tile.TileContext(nc))
        dram = stack.enter_context(tc.tile_pool(name="dram", bufs=1, space="DRAM"))
        const = stack.enter_context(tc.tile_pool(name="const", bufs=1))
        ps = stack.enter_context(tc.tile_pool(name="ps", bufs=1, space="PSUM"))

        ident = const.tile([128, 128], BF, tag="ident", bufs=1)
        make_identity(nc, ident[:])

        rs_in = dram.tile([1, OL], F32, tag="rs_in", bufs=1)
        rs_out = dram.tile([1, OL], F32, tag="rs_out", bufs=1)
        tmp_in = [dram.tile([OL, DH], ar_dtype, tag=f"tmp_in{h}", bufs=1,
                            name=f"tmp_in{h}")
                  for h in range(2)]
        tmp_out = [dram.tile([OL, DH], ar_dtype, tag=f"tmp_out{h}", bufs=1,
                             name=f"tmp_out{h}")
                   for h in range(2)]
        x_bf_dram = dram.tile([D, NL], BF, tag="x_bf_dram", bufs=1)

        # mean_neg_row[t, :] = -rowmean(Psi) for o-tile t (bf16)
        mean_neg_row = const.tile([1, OL], BF, tag="mean_neg_row", bufs=1)
        xrs_bf = const.tile([1, D], BF, tag="xrs_bf", bufs=1)

        # ============ phase A + mm1 scope ============
        with tc.tile_pool(name="sb1", bufs=1) as sb:
            x_bf = {}

            def x_chunk_load(dc):
                for dt in range(4):
                    xb = sb.tile([128, NL], BF, tag="x_bf", bufs=6,
                                 name=f"x_bf{dc}_{dt}")
                    x_bf[(dc, dt)] = xb
                    row = dc * 512 + dt * 128
                    nc.gpsimd.dma_start(xb[:], x_ext[row: row + 128, :])
                    # natural-layout bf16 copy for mm2 streaming
                    nc.scalar.dma_start(x_bf_dram[row: row + 128, :], xb[:])

            def x_chunk_transpose(dc, xT, k_lo, k_hi):
                for k in range(k_lo, k_hi):
                    pt = ps.tile([128, 512], BF, tag="pst", bufs=2,
                                 name=f"pstx{dc}_{k}")
                    for dt in range(4):
                        nc.tensor.transpose(
                            pt[:, dt * 128:(dt + 1) * 128],
                            x_bf[(dc, dt)][:, k * 128:(k + 1) * 128],
                            ident[:])
                    nc.vector.tensor_copy(xT[:, k * 512:(k + 1) * 512], pt[:])

            x_chunk_load(0)
            nc.scalar.dma_start(rs_in[:], rs_ext[:])
            nc.gpsimd.collective_compute(
                "AllReduce", mybir.AluOpType.add, replica_groups=groups,
                ins=[rs_in.opt()], outs=[rs_out.opt()])
            psi_bf = []
            for t in range(OT):
                pb = sb.tile([128, NL], BF, tag="psi_bf", bufs=OT,
                             name=f"psi_bf{t}")
                psi_bf.append(pb)
                nc.gpsimd.dma_start(pb[:], psi_ext[t * 128:(t + 1) * 128, :])
            nc.gpsimd.dma_start(xrs_bf[:], xrs_ext[:])
            mn_f = sb.tile([1, OL], F32, tag="mn_f", bufs=1)
            nc.scalar.dma_start(mn_f[:], rs_out[:])
            nc.vector.tensor_scalar_mul(mn_f[:], mn_f[:], -1.0 / NTOT)
            nc.vector.tensor_copy(mean_neg_row[:], mn_f[:])

            # psiT: block k at cols [k*OL, (k+1)*OL) = [128(n), OL(o)] bf16
            psiT = sb.tile([128, KN * OL], BF, tag="psiT", bufs=1)
            xT_bufs = [sb.tile([128, KN * 512], BF, tag="xT", bufs=2,
                               name=f"xT{b}")
                       for b in range(2)]

            # chunk-0 transposes + uncentered psi transposes (no AR dep)
            x_chunk_load(1)
            x_chunk_transpose(0, xT_bufs[0], 0, KN)
            for k in range(KN):
                pt = ps.tile([128, OL], BF, tag="pst", bufs=2, name=f"pstp{k}")
                for t in range(OT):
                    nc.tensor.transpose(pt[:, t * 128:(t + 1) * 128],
                                        psi_bf[t][:, k * 128:(k + 1) * 128],
                                        ident[:])
                nc.vector.tensor_copy(psiT[:, k * OL:(k + 1) * OL], pt[:])

            # ---- mm1 ----
            for dc in range(DC):
                xT = xT_bufs[dc % 2]
                if dc + 2 < DC:
                    x_chunk_load(dc + 2)
                mm = [ps.tile([128, 512], F32, tag="mmps", bufs=6,
                              name=f"mm1_{dc}_{_ot}")
                      for _ot in range(OT)]
                for ot in range(OT):
                    for k in range(KN):
                        nc.tensor.matmul(
                            mm[ot][:],
                            psiT[:, k * OL + ot * 128: k * OL + (ot + 1) * 128],
                            xT[:, k * 512:(k + 1) * 512],
                            start=(k == 0), stop=False)
                    # rank-1 centering correction: tmp -= mean[o] * xrs[d]
                    for q in range(4):
                        nc.tensor.matmul(
                            mm[ot][:, q * 128:(q + 1) * 128],
                            mean_neg_row[0:1, ot * 128:(ot + 1) * 128],
                            xrs_bf[0:1, dc * 512 + q * 128:
                                   dc * 512 + (q + 1) * 128],
                            start=False, stop=(q == 3))
                    # interleave next chunk's transposes between mm groups
                    if dc + 1 < DC:
                        x_chunk_transpose(dc + 1, xT_bufs[(dc + 1) % 2],
                                          ot * (KN // OT),
                                          (ot + 1) * (KN // OT))
                h, dci = dc // (DC // 2), dc % (DC // 2)
                for ot in range(OT):
                    stage = sb.tile([128, 512], ar_dtype, tag="t1stage", bufs=8,
                                    name=f"t1stage{dc}_{ot}")
                    nc.vector.tensor_copy(stage[:], mm[ot][:])
                    nc.scalar.dma_start(
                        tmp_in[h][ot * 128:(ot + 1) * 128,
                                  dci * 512:(dci + 1) * 512],
                        stage[:])
                if dc == DC // 2 - 1:
                    nc.gpsimd.collective_compute(
                        "AllReduce", mybir.AluOpType.add, replica_groups=groups,
                        ins=[tmp_in[0].opt()], outs=[tmp_out[0].opt()])
            nc.gpsimd.collective_compute(
                "AllReduce", mybir.AluOpType.add, replica_groups=groups,
                ins=[tmp_in[1].opt()], outs=[tmp_out[1].opt()])

        # ============ mm2 scope ============
        with tc.tile_pool(name="sb2", bufs=1) as sb:
            tmpT = sb.tile([128, KD * OL], BF, tag="tmpT", bufs=1)
            out_part = [sb.tile([128, NL], F32, tag=f"out_part{ot}", bufs=1,
                                name=f"out_part{ot}")
                        for ot in range(OT)]
            for h in range(2):
                tmp_sb = []
                for t in range(OT):
                    tl = sb.tile([128, DH], ar_dtype, tag="tmp_sb", bufs=OT,
                                 name=f"tmp_sb{h}_{t}")
                    nc.scalar.dma_start(tl[:], tmp_out[h][t * 128:(t + 1) * 128, :])
                    tmp_sb.append(tl)
                for kdl in range(KD // 2):
                    kd = h * (KD // 2) + kdl
                    pt = ps.tile([128, OL], BF, tag="pst", bufs=2,
                                 name=f"pst2_{kd}")
                    for t in range(OT):
                        nc.tensor.transpose(pt[:, t * 128:(t + 1) * 128],
                                            tmp_sb[t][:, kdl * 128:(kdl + 1) * 128],
                                            ident[:])
                    nc.vector.tensor_copy(tmpT[:, kd * OL:(kd + 1) * OL], pt[:])
                for ncn in range(ND):
                    mm = [ps.tile([128, 512], F32, tag="mmps", bufs=6,
                                  name=f"mm2_{h}_{ncn}_{_ot}")
                          for _ot in range(OT)]
                    for kdl in range(KD // 2):
                        kd = h * (KD // 2) + kdl
                        x2b = sb.tile([128, 512], BF, tag="x2b", bufs=8,
                                      name=f"x2b{h}_{ncn}_{kdl}")
                        dma_eng = nc.sync if (kdl % 2 == 0) else nc.scalar
                        dma_eng.dma_start(
                            x2b[:], x_bf_dram[kd * 128:(kd + 1) * 128,
                                              ncn * 512:(ncn + 1) * 512])
                        for ot in range(OT):
                            nc.tensor.matmul(
                                mm[ot][:],
                                tmpT[:, kd * OL + ot * 128: kd * OL + (ot + 1) * 128],
                                x2b[:],
                                start=(kdl == 0), stop=(kdl == KD // 2 - 1))
                    for ot in range(OT):
                        if h == 0:
                            nc.vector.tensor_copy(
                                out_part[ot][:, ncn * 512:(ncn + 1) * 512],
                                mm[ot][:])
                        else:
                            ostage = sb.tile([128, 512], F32, tag="ostage",
                                             bufs=8, name=f"ostage{ncn}_{ot}")
                            nc.vector.tensor_tensor(
                                ostage[:], mm[ot][:],
                                out_part[ot][:, ncn * 512:(ncn + 1) * 512],
                                op=mybir.AluOpType.add)
                            nc.scalar.dma_start(
                                out_ext[ot * 128:(ot + 1) * 128,
                                        ncn * 512:(ncn + 1) * 512],
                                ostage[:])
    nc.compile()
    return nc


def make_in_maps(x, Psi, n_cores=8, NL=4096, OL=512):
    """Shard full inputs for the 2x4 grid, with host-side row-sum stats."""
    import numpy as np
    OT = OL // 128
    in_maps = []
    for c in range(n_cores):
        i, j = c % 2, c // 2
        xs = np.ascontiguousarray(x[:, i * NL:(i + 1) * NL])
        ps_ = np.ascontiguousarray(Psi[j * OL:(j + 1) * OL, i * NL:(i + 1) * NL])
        in_maps.append({
            "x": xs,
            "psi": ps_,
            "rs": ps_.sum(axis=1, dtype=np.float64).astype(np.float32).reshape(1, -1),
            "xrs": xs.sum(axis=1, dtype=np.float64).astype(np.float32).reshape(1, -1),
        })
    return in_maps


# ---------------- harness-facing wrapper ----------------
import numpy as np

_NC_CACHE = {}

D_FULL, N_FULL, O_FULL = 4096, 8192, 2048
NL_, OL_ = 4096, 512
N_CORES = 8
GROUPS = ((0, 1), (2, 3), (4, 5), (6, 7))


def _get_nc():
    if "nc" not in _NC_CACHE:
        _NC_CACHE["nc"] = build_srp_kernel(
            D=D_FULL, NL=NL_, OL=OL_, NTOT=N_FULL,
            n_cores=N_CORES, groups=GROUPS)
    return _NC_CACHE["nc"]


def kernel(x, Psi):
    """out = (Psi - rowmean(Psi)) @ x.T @ x on 8 TRN2 NeuronCores."""
    from concourse.bass_utils import run_bass_kernel_spmd
    x = np.asarray(x, dtype=np.float32)
    Psi = np.asarray(Psi, dtype=np.float32)
    assert x.shape == (D_FULL, N_FULL) and Psi.shape == (O_FULL, N_FULL)
    nc = _get_nc()
    in_maps = make_in_maps(x, Psi, n_cores=N_CORES, NL=NL_, OL=OL_)
    res = run_bass_kernel_spmd(nc, in_maps, core_ids=list(range(N_CORES)))
    out = np.empty((O_FULL, N_FULL), dtype=np.float32)
    for c in range(N_CORES):
        i, j = c % 2, c // 2
        out[j * OL_:(j + 1) * OL_, i * NL_:(i + 1) * NL_] = res.results[c]["out"]
    return out



# revision 3
# speedup vs baseline: 1.1483x; 1.1483x over previous
"""SRP layer distributed Bass kernel for TRN2 (v7).

Math (full problem): out = Psi_c @ x.T @ x with Psi_c = Psi - rowmean(Psi).
  x [D, N] f32, Psi [O, N] f32, out [O, N] f32  (D=4096, N=8192, O=2048)

Distribution over 8 cores as a 4x2 grid: core c -> (i = c % 4: n-quarter,
j = c // 4: o-half). The host pre-centers Psi (global row-mean), pre-slices,
pre-transposes, and pre-casts to bf16, so the device does NOTHING but the
two GEMMs and the tmp AllReduce:

Per core (NL = N/4 = 2048, OL = O/2 = 1024):
  xT   [NL, D]  bf16  (x_i.T)        - mm1 stationary operand
  x    [D, NL]  bf16  (x_i)          - mm2 moving operand
  psiT [NL, OL] bf16  (Psi_c_ji.T)   - mm1 moving operand
  out  [OL, NL] f32

mm1: tmpT[d, o] = sum_n xT[n, d] * psiT[n, o]   (partial over local n)
     -> bf16 -> DRAM in 4 d-quarters, each AllReduce'd over the 4 cores
     of the same o-half as soon as it is ready (overlaps mm1 tail + mm2).
mm2: out[o, n] = sum_d tmpT[d, o] * x[d, n], two kd-half passes so pass A
     (kd 0..15) runs while quarters 2,3 still AllReduce; pass B adds and
     streams the f32 result out.

No PE transposes, no centering ops: PE does exactly 2048 [128x128x512]
bf16 matmuls (~437 us at peak) plus nothing else.
"""

from contextlib import ExitStack

import concourse.bacc as bacc
import concourse.mybir as mybir
import concourse.tile as tile

F32 = mybir.dt.float32
BF = mybir.dt.bfloat16


def build_srp_kernel(
    D=4096,
    NL=2048,
    OL=1024,
    n_cores=8,
    groups=((0, 1, 2, 3), (4, 5, 6, 7)),
):
    DT = D // 128    # 32 d-tiles (tmpT partition tiles / mm2 contraction)
    NT = NL // 128   # 16 n-tiles (mm1 contraction)
    OC = OL // 512   # 2  o-chunks (mm1 free cols)
    NCH = NL // 512  # 4  n-chunks (mm2 free cols)
    OT = OL // 128   # 8  o-tiles (mm2 output partition tiles)
    DC = D // 512    # 8  xT d-chunks (streamed)
    NQ = 4           # AllReduce chunks (d-quarters of tmpT)
    DQ = DT // NQ    # 8 d-tiles per AR quarter
    KH = DT // 2     # 16 kd per mm2 pass

    groups = [list(g) for g in groups]

    nc = bacc.Bacc("TRN2", target_bir_lowering=False, debug=False,
                   num_devices=n_cores)
    xT_ext = nc.dram_tensor("xT", [NL, D], BF, kind="ExternalInput")
    x_ext = nc.dram_tensor("x", [D, NL], BF, kind="ExternalInput")
    psiT_ext = nc.dram_tensor("psiT", [NL, OL], BF, kind="ExternalInput")
    out_ext = nc.dram_tensor("out", [OL, NL], F32, kind="ExternalOutput")

    with ExitStack() as stack:
        tc = stack.enter_context(tile.TileContext(nc))
        dram = stack.enter_context(tc.tile_pool(name="dram", bufs=1, space="DRAM"))
        ps = stack.enter_context(tc.tile_pool(name="ps", bufs=1, space="PSUM"))
        sbl = stack.enter_context(tc.tile_pool(name="sbl", bufs=1))

        tmp_in = [dram.tile([DQ * 128, OL], BF, tag=f"tmp_in{q}", bufs=1,
                            name=f"tmp_in{q}") for q in range(NQ)]
        tmp_out = [dram.tile([DQ * 128, OL], BF, tag=f"tmp_out{q}", bufs=1,
                             name=f"tmp_out{q}") for q in range(NQ)]

        # ============ mm1 ============
        with tc.tile_pool(name="sb1", bufs=1) as sb1:
            psiT_sb = []
            for ntt in range(NT):
                pb = sb1.tile([128, OL], BF, tag="psiT", bufs=NT,
                              name=f"psiT{ntt}")
                nc.scalar.dma_start(pb[:], psiT_ext[ntt * 128:(ntt + 1) * 128, :])
                psiT_sb.append(pb)

            xtc = {}

            def load_chunk(dc):
                for ntt in range(NT):
                    t = sb1.tile([128, 512], BF, tag="xTc", bufs=3 * NT,
                                 name=f"xTc{dc}_{ntt}")
                    nc.gpsimd.dma_start(
                        t[:], xT_ext[ntt * 128:(ntt + 1) * 128,
                                     dc * 512:(dc + 1) * 512])
                    xtc[(dc, ntt)] = t

            load_chunk(0)
            load_chunk(1)
            load_chunk(2)

            for dt in range(DT):
                dc = dt // 4
                if dt % 4 == 0 and dc + 3 < DC:
                    load_chunk(dc + 3)
                mm = [ps.tile([128, 512], F32, tag="mm1", bufs=4,
                              name=f"mm1_{dt}_{_oc}") for _oc in range(OC)]
                doff = (dt % 4) * 128
                for ntt in range(NT):
                    lhs = xtc[(dc, ntt)]
                    for oc in range(OC):
                        nc.tensor.matmul(
                            mm[oc][:],
                            lhs[:, doff:doff + 128],
                            psiT_sb[ntt][:, oc * 512:(oc + 1) * 512],
                            start=(ntt == 0), stop=(ntt == NT - 1))
                q, dq = dt // DQ, dt % DQ
                for oc in range(OC):
                    st = sb1.tile([128, 512], BF, tag="t1s", bufs=4,
                                  name=f"t1s{dt}_{oc}")
                    nc.vector.tensor_copy(st[:], mm[oc][:])
                    nc.scalar.dma_start(
                        tmp_in[q][dq * 128:(dq + 1) * 128,
                                  oc * 512:(oc + 1) * 512],
                        st[:])
                if dt % DQ == DQ - 1:
                    nc.gpsimd.collective_compute(
                        "AllReduce", mybir.AluOpType.add,
                        replica_groups=groups,
                        ins=[tmp_in[q].opt()], outs=[tmp_out[q].opt()])

            # mm2 input loads, emitted inside sb1's scope but living in sbl.
            # x2b tiles are consumed ncn-major (mm2's outer loop) so each
            # (kd, ncn) tile dies within one ncn iteration; bufs=3*KH keeps
            # three ncn-sets in flight without WAR cycles. sync queue order:
            # early x2b prefetch, then AR-gated tmp_sb quarters, then the
            # rest of x2b (throttled by buffer rotation).
            tmp_sb = {}

            def load_tmp_q(q):
                for dq in range(DQ):
                    kd = q * DQ + dq
                    t = sbl.tile([128, OL], BF, tag="tmp_sb", bufs=DT,
                                 name=f"tmp_sb{kd}")
                    nc.sync.dma_start(t[:], tmp_out[q][dq * 128:(dq + 1) * 128, :])
                    tmp_sb[kd] = t

            x2b = {}

            def load_x2b(p, ncn):
                for kd in range(p * KH, (p + 1) * KH):
                    t = sbl.tile([128, 512], BF, tag="x2b", bufs=3 * KH,
                                 name=f"x2b{kd}_{ncn}")
                    nc.sync.dma_start(
                        t[:], x_ext[kd * 128:(kd + 1) * 128,
                                    ncn * 512:(ncn + 1) * 512])
                    x2b[(kd, ncn)] = t

            load_x2b(0, 0)
            load_x2b(0, 1)
            load_x2b(0, 2)
            load_tmp_q(0)
            load_tmp_q(1)
            load_tmp_q(2)
            load_tmp_q(3)
            load_x2b(0, 3)
            for ncn in range(NCH):
                load_x2b(1, ncn)

        # ============ mm2 ============
        with tc.tile_pool(name="sb2", bufs=1) as sb2:
            out_part = [sb2.tile([128, NL], F32, tag="out_part", bufs=OT,
                                 name=f"out_part{ot}") for ot in range(OT)]
            for p in range(2):
                for ncn in range(NCH):
                    for ot in range(OT):
                        mmo = ps.tile([128, 512], F32, tag="mm2", bufs=4,
                                      name=f"mm2_{p}_{ncn}_{ot}")
                        for dq in range(KH):
                            kd = p * KH + dq
                            nc.tensor.matmul(
                                mmo[:],
                                tmp_sb[kd][:, ot * 128:(ot + 1) * 128],
                                x2b[(kd, ncn)][:],
                                start=(dq == 0), stop=(dq == KH - 1))
                        if p == 0:
                            nc.vector.tensor_copy(
                                out_part[ot][:, ncn * 512:(ncn + 1) * 512],
                                mmo[:])
                        else:
                            ost = sb2.tile([128, 512], F32, tag="ost", bufs=4,
                                           name=f"ost{ot}_{ncn}")
                            nc.vector.tensor_tensor(
                                ost[:], mmo[:],
                                out_part[ot][:, ncn * 512:(ncn + 1) * 512],
                                op=mybir.AluOpType.add)
                            nc.scalar.dma_start(
                                out_ext[ot * 128:(ot + 1) * 128,
                                        ncn * 512:(ncn + 1) * 512],
                                ost[:])
    nc.compile()
    return nc


def make_in_maps(x, Psi, n_cores=8, NL=2048, OL=1024):
    """Shard full f32 inputs for the 4x2 grid with host-side prep:
    center Psi with the global row-mean, slice, transpose, cast bf16."""
    import numpy as np
    import ml_dtypes
    bf16 = ml_dtypes.bfloat16

    Psi_c = (Psi.astype(np.float64)
             - Psi.mean(axis=1, dtype=np.float64, keepdims=True))
    in_maps = []
    for c in range(n_cores):
        i, j = c % 4, c // 4
        xs = x[:, i * NL:(i + 1) * NL].astype(np.float32)
        ps_ = Psi_c[j * OL:(j + 1) * OL, i * NL:(i + 1) * NL]
        in_maps.append({
            "x": np.ascontiguousarray(xs).astype(bf16),
            "xT": np.ascontiguousarray(xs.T).astype(bf16),
            "psiT": np.ascontiguousarray(ps_.T).astype(bf16),
        })
    return in_maps


# ---------------- harness-facing wrapper ----------------
import numpy as np

_NC_CACHE = {}

D_FULL, N_FULL, O_FULL = 4096, 8192, 2048
NL_, OL_ = 2048, 1024
N_CORES = 8
GROUPS = ((0, 1, 2, 3), (4, 5, 6, 7))


def _get_nc():
    if "nc" not in _NC_CACHE:
        _NC_CACHE["nc"] = build_srp_kernel(
            D=D_FULL, NL=NL_, OL=OL_, n_cores=N_CORES, groups=GROUPS)
    return _NC_CACHE["nc"]


def kernel(x, Psi):
    """out = (Psi - rowmean(Psi)) @ x.T @ x on 8 TRN2 NeuronCores."""
    from concourse.bass_utils import run_bass_kernel_spmd
    x = np.asarray(x, dtype=np.float32)
    Psi = np.asarray(Psi, dtype=np.float32)
    assert x.shape == (D_FULL, N_FULL) and Psi.shape == (O_FULL, N_FULL)
    nc = _get_nc()
    in_maps = make_in_maps(x, Psi, n_cores=N_CORES, NL=NL_, OL=OL_)
    res = run_bass_kernel_spmd(nc, in_maps, core_ids=list(range(N_CORES)))
    out = np.empty((O_FULL, N_FULL), dtype=np.float32)
    for c in range(N_CORES):
        i, j = c % 4, c // 4
        out[j * OL_:(j + 1) * OL_, i * NL_:(i + 1) * NL_] = res.results[c]["out"]
    return out
